# revision 1
# baseline (speedup 1.0000x reference)
"""DGCNN Bass kernel for trn2 — per-core builder + host-side folding.

Per core (one sample, N points, k=40 neighbors):
  1. kNN scores via K=4 matmul (s_ij = x_i.x_j - 0.5|x_j|^2; row-affine
     equivalent to the reference's pairwise -dist^2).
  2. top-40 selection on DVE (max8 / max_index / match_replace rounds).
  3. EdgeConv rounds with gather-after-matmul factorization:
     conv([nbr-ctr, ctr]) = A[:, j] + B[:, i], A/B per-point tables.
  4. Global-max head with W7 split (g-part reduces to a per-channel bias).

BN scales folded into weights on host; LeakyReLU commutes with the k/N max
reductions (positive BN scale asserted host-side).
"""
import numpy as np
import concourse.bass as bass
import concourse.mybir as mybir
from concourse.bacc import Bacc
from concourse.tile import TileContext

F32 = mybir.dt.float32
BF16 = mybir.dt.bfloat16
U16 = mybir.dt.uint16
I16 = mybir.dt.int16
AX = mybir.AxisListType
OP = mybir.AluOpType
ACTF = mybir.ActivationFunctionType

KNBR = 40
NEG = -1e30
LEAK = 0.2


def build_core(N=4096, conv_dtype="bf16"):
    nc = Bacc(None)
    T = N // 128
    PAIRS = T // 2
    CD = {"f32": F32, "bf16": BF16}[conv_dtype]

    def din(name, shape, dt=F32):
        return nc.dram_tensor(name, shape, dt, kind="ExternalInput")

    xr_d = din("xr", [4, N])
    xa_d = din("xa", [4, N])
    a1w_d = din("a1w", [3, 64])
    b1w_d = din("b1w", [3, 64])
    b1_d = din("b1", [64, 1])
    w2t_d = din("w2t", [64, 64], CD)
    b2_d = din("b2", [64, 1])
    a3w_d = din("a3w", [64, 64])
    b3w_d = din("b3w", [64, 64])
    b3_d = din("b3", [64, 1])
    w4t_d = din("w4t", [64, 64], CD)
    b4_d = din("b4", [64, 1])
    a5w_d = din("a5w", [64, 64])
    b5w_d = din("b5w", [64, 64])
    b5_d = din("b5", [64, 1])
    w6t_d = din("w6t", [64, 3 * 1024])
    b6_d = din("b6", [128, 8])
    w7gt_d = din("w7gt", [128, 8 * 4 * 128])
    b7_d = din("b7", [128, 4])
    w7xt_d = din("w7xt", [64, 3 * 4 * 128])
    w8t_d = din("w8t", [128, 4 * 2 * 128], CD)
    b8_d = din("b8", [128, 2])
    w9t_d = din("w9t", [128, 2 * 63], CD)
    b9_d = din("b9", [63, 1])

    out_d = nc.dram_tensor("out", [63, N], F32, kind="ExternalOutput")
    debug = bool(int(__import__("os").environ.get("DGCNN_DEBUG", "0")))
    if debug:
        idx_dbg = nc.dram_tensor("idx_dbg", [128, T * KNBR], U16, kind="ExternalOutput")
        x1_dbg = nc.dram_tensor("x1_dbg", [64, N], F32, kind="ExternalOutput")
        x2_dbg = nc.dram_tensor("x2_dbg", [64, N], F32, kind="ExternalOutput")
        x3_dbg = nc.dram_tensor("x3_dbg", [64, N], F32, kind="ExternalOutput")
        g_dbg = nc.dram_tensor("g_dbg", [128, 8], F32, kind="ExternalOutput")

    with TileContext(nc) as tc:
        with tc.tile_pool(name="persist", bufs=1) as pp:
            # per-tile wrapped edge lists; tile t lives at partitions
            # {0-15, 32-47} (t even) or {64-79, 96-111} (t odd), cols t*320..
            wraps = pp.tile([128, 320 * T], U16)
            x1 = pp.tile([64, N], F32)
            x2 = pp.tile([64, N], F32)
            x3 = pp.tile([64, N], F32)
            nbr_all = pp.tile([128, T * KNBR], U16)

            # =====================================================
            # Stage A: kNN + top-40 per tile (monolithic rounds)
            # (ec1 pool + xp pool opened around it: LIFO scoping)
            # =====================================================
            ec1p = tc.tile_pool(name="ec1", bufs=1)
            ecp1 = ec1p.__enter__()
            a1rep = ecp1.tile([128, N], F32, name="a1rep")
            b1rep = ecp1.tile([128, N], F32, name="b1rep")
            w2t = ecp1.tile([128, 64], CD, name="w2t")
            nc.sync.dma_start(out=w2t[0:64, :], in_=w2t_d[:])
            nc.sync.dma_start(out=w2t[64:128, :], in_=w2t_d[:])
            xp = tc.tile_pool(name="xp", bufs=1)
            xpp = xp.__enter__()
            xr = xpp.tile([4, N], F32, name="xr")
            nc.sync.dma_start(out=xr[:], in_=xr_d[:])
            xa = xpp.tile([4, N], F32, name="xa")
            nc.sync.dma_start(out=xa[:], in_=xa_d[:])
            with tc.tile_pool(name="sel_sb", bufs=2) as sp, \
                 tc.tile_pool(name="sel_ps", bufs=2, space="PSUM") as sps:
                for t in range(T):
                    s_sb = sp.tile([128, N], F32, tag="s_sb")
                    for h in range(2):
                        ps = sps.tile([128, N // 2], F32, tag="score")
                        for j in range(N // 2 // 512):
                            col = h * (N // 2) + j * 512
                            nc.tensor.matmul(
                                ps[:, j * 512:(j + 1) * 512],
                                xa[:, t * 128:(t + 1) * 128],
                                xr[:, col:col + 512],
                                start=True, stop=True)
                        nc.scalar.copy(out=s_sb[:, h * (N // 2):(h + 1) * (N // 2)],
                                       in_=ps[:])
                    m8 = sp.tile([128, 8], F32, tag="m8")
                    for r in range(5):
                        nc.vector.max(out=m8[:], in_=s_sb[:])
                        nc.vector.max_index(
                            out=nbr_all[:, t * KNBR + r * 8: t * KNBR + (r + 1) * 8],
                            in_max=m8[:], in_values=s_sb[:])
                        nc.vector.match_replace(out=s_sb[:], in_to_replace=m8[:],
                                                in_values=s_sb[:], imm_value=NEG)
                    # wrapped list: wrap[ilo, kk*8+ihi] = nbr[ihi*16+ilo, kk]
                    pbase = 64 * (t % 2)
                    for ihi in range(8):
                        dst = wraps[pbase:pbase + 16, t * 320:(t + 1) * 320] \
                            .rearrange("p (k e) -> p k e", e=8)[:, :, ihi:ihi + 1]
                        nc.sync.dma_start(
                            out=dst,
                            in_=nbr_all[ihi * 16:(ihi + 1) * 16,
                                        t * KNBR:(t + 1) * KNBR].unsqueeze(2))
                    # replicate to the second quadrant (+32 partitions)
                    nc.sync.dma_start(
                        out=wraps[pbase + 32: pbase + 48, t * 320:(t + 1) * 320],
                        in_=wraps[pbase: pbase + 16, t * 320:(t + 1) * 320])
            if debug:
                nc.sync.dma_start(out=idx_dbg[:], in_=nbr_all[:])

            # =====================================================
            # EdgeConv machinery
            # =====================================================
            def build_tables(aw_d, bw_d, bias_d, src, arep, brep, kdim):
                with tc.tile_pool(name="tb_sb", bufs=2) as tsp, \
                     tc.tile_pool(name="tb_ps", bufs=2, space="PSUM") as tps:
                    awt = tsp.tile([kdim, 64], F32, tag="awt")
                    bwt = tsp.tile([kdim, 64], F32, tag="bwt")
                    biast = tsp.tile([64, 1], F32, tag="biast")
                    nc.sync.dma_start(out=awt[:], in_=aw_d[:])
                    nc.sync.dma_start(out=bwt[:], in_=bw_d[:])
                    nc.sync.dma_start(out=biast[:], in_=bias_d[:])
                    for j in range(N // 512):
                        psa = tps.tile([64, 512], F32, tag="psa")
                        nc.tensor.matmul(psa[:], awt[:], src[:, j * 512:(j + 1) * 512],
                                         start=True, stop=True)
                        nc.scalar.copy(out=arep[0:64, j * 512:(j + 1) * 512], in_=psa[:])
                        psb = tps.tile([64, 512], F32, tag="psb")
                        nc.tensor.matmul(psb[:], bwt[:], src[:, j * 512:(j + 1) * 512],
                                         start=True, stop=True)
                        nc.scalar.activation(brep[0:64, j * 512:(j + 1) * 512], psb[:],
                                             ACTF.Identity, bias=biast[:])
                    nc.sync.dma_start(out=arep[64:128, :], in_=arep[0:64, :])
                    nc.sync.dma_start(out=brep[64:128, 0:N - 128],
                                      in_=brep[0:64, 128:N])

            def edge_round(arep, brep, wt_t, bias_t, xout, last=False):
                with tc.tile_pool(name="er_sb", bufs=2) as esp, \
                     tc.tile_pool(name="er_ps", bufs=2, space="PSUM") as eps:
                    for p in range(PAIRS):
                        tA = 2 * p
                        idx128 = esp.tile([128, 320], U16, tag="idx128")
                        rep16 = [i % 16 for i in range(32)]
                        nc.vector.stream_shuffle(
                            idx128[0:64, :],
                            wraps[0:64, tA * 320:(tA + 1) * 320], mask=rep16)
                        nc.vector.stream_shuffle(
                            idx128[64:128, :],
                            wraps[64:128, (tA + 1) * 320:(tA + 2) * 320], mask=rep16)
                        ga = esp.tile([128, KNBR * 128], F32, tag="ga")
                        nc.gpsimd.ap_gather(ga[:], arep[:], idx128.bitcast(I16),
                                            channels=128, num_elems=N, d=1,
                                            num_idxs=KNBR * 128)
                        if last:
                            mx = esp.tile([128, 128], F32, tag="mx")
                            nc.vector.tensor_reduce(
                                out=mx[:], in_=ga.rearrange("p (k i) -> p i k", k=KNBR),
                                axis=AX.X, op=OP.max)
                            zz = esp.tile([128, 128], F32, tag="zz")
                            nc.vector.tensor_tensor(
                                out=zz[:], in0=mx[:],
                                in1=brep[:, tA * 128: tA * 128 + 128], op=OP.add)
                            xo = esp.tile([128, 128], F32, tag="xo")
                            nc.vector.scalar_tensor_tensor(
                                out=xo[:], in0=zz[:], scalar=LEAK, in1=zz[:],
                                op0=OP.mult, op1=OP.max)
                        else:
                            bview = brep[:, tA * 128: tA * 128 + 128] \
                                .unsqueeze(1).broadcast_to([128, KNBR, 128])
                            e = esp.tile([128, KNBR * 128], CD, tag="e")
                            nc.vector.tensor_tensor(
                                out=e.rearrange("p (k i) -> p k i", k=KNBR),
                                in0=ga.rearrange("p (k i) -> p k i", k=KNBR),
                                in1=bview, op=OP.add)
                            nc.vector.scalar_tensor_tensor(
                                out=e[:], in0=e[:], scalar=LEAK, in1=e[:],
                                op0=OP.mult, op1=OP.max)
                            NCH = KNBR * 128 // 512
                            pmax = esp.tile([128, NCH * 128], F32, tag="pmax")
                            for c in range(NCH):
                                cps = eps.tile([128, 512], F32, tag="cps")
                                nc.tensor.matmul(cps[0:64, :], wt_t[0:64, :],
                                                 e[0:64, c * 512:(c + 1) * 512],
                                                 start=True, stop=True)
                                nc.tensor.matmul(cps[64:128, :], wt_t[64:128, :],
                                                 e[64:128, c * 512:(c + 1) * 512],
                                                 start=True, stop=True)
                                nc.vector.tensor_reduce(
                                    out=pmax[:, c * 128:(c + 1) * 128],
                                    in_=cps.rearrange("p (k i) -> p i k", k=4),
                                    axis=AX.X, op=OP.max)
                            mx = esp.tile([128, 128], F32, tag="mx")
                            nc.vector.tensor_reduce(
                                out=mx[:], in_=pmax.rearrange("p (k i) -> p i k", k=NCH),
                                axis=AX.X, op=OP.max)
                            xo = esp.tile([128, 128], F32, tag="xo")
                            nc.scalar.activation(xo[:], mx[:], ACTF.Identity,
                                                 bias=bias_t[:])
                            nc.vector.scalar_tensor_tensor(
                                out=xo[:], in0=xo[:], scalar=LEAK, in1=xo[:],
                                op0=OP.mult, op1=OP.max)
                        nc.sync.dma_start(out=xout[:, tA * 128:(tA + 1) * 128],
                                          in_=xo[0:64, :])
                        nc.sync.dma_start(out=xout[:, (tA + 1) * 128:(tA + 2) * 128],
                                          in_=xo[64:128, :])

            def load_bias128(bias_d_, pool):
                bt = pool.tile([128, 1], F32, tag="bias128")
                nc.sync.dma_start(out=bt[0:64, :], in_=bias_d_[:])
                nc.sync.dma_start(out=bt[64:128, :], in_=bias_d_[:])
                return bt

            # ---- EdgeConv 1 ----
            build_tables(a1w_d, b1w_d, b1_d, xa[0:3, :], a1rep, b1rep, 3)
            xp.__exit__(None, None, None)
            b2r = load_bias128(b2_d, ecp1)
            edge_round(a1rep, b1rep, w2t, b2r, x1)
            ec1p.__exit__(None, None, None)

            # ---- EdgeConv 2 ----
            with tc.tile_pool(name="ec2", bufs=1) as ecp:
                a3rep = ecp.tile([128, N], F32)
                b3rep = ecp.tile([128, N], F32)
                w4t = ecp.tile([128, 64], CD)
                nc.sync.dma_start(out=w4t[0:64, :], in_=w4t_d[:])
                nc.sync.dma_start(out=w4t[64:128, :], in_=w4t_d[:])
                b4r = load_bias128(b4_d, ecp)
                build_tables(a3w_d, b3w_d, b3_d, x1, a3rep, b3rep, 64)
                edge_round(a3rep, b3rep, w4t, b4r, x2)

            # ---- EdgeConv 3 ----
            with tc.tile_pool(name="ec3", bufs=1) as ecp:
                a5rep = ecp.tile([128, N], F32)
                b5rep = ecp.tile([128, N], F32)
                build_tables(a5w_d, b5w_d, b5_d, x2, a5rep, b5rep, 64)
                edge_round(a5rep, b5rep, None, None, x3, last=True)

            # =====================================================
            # Head
            # =====================================================
            with tc.tile_pool(name="hd", bufs=1) as hp, \
                 tc.tile_pool(name="hd_sb", bufs=3) as hsp, \
                 tc.tile_pool(name="hd_ps", bufs=2, space="PSUM") as hps:
                w6t = hp.tile([64, 3 * 1024], F32)
                nc.sync.dma_start(out=w6t[:], in_=w6t_d[:])
                b6t = hp.tile([128, 8], F32)
                nc.sync.dma_start(out=b6t[:], in_=b6_d[:])
                w7gt = hp.tile([128, 8 * 4 * 128], F32)
                nc.sync.dma_start(out=w7gt[:], in_=w7gt_d[:])
                b7t = hp.tile([128, 4], F32)
                nc.sync.dma_start(out=b7t[:], in_=b7_d[:])
                w7xt = hp.tile([64, 3 * 4 * 128], F32)
                nc.sync.dma_start(out=w7xt[:], in_=w7xt_d[:])
                w8t = hp.tile([128, 4 * 2 * 128], CD)
                nc.sync.dma_start(out=w8t[:], in_=w8t_d[:])
                b8t = hp.tile([128, 2], F32)
                nc.sync.dma_start(out=b8t[:], in_=b8_d[:])
                w9t = hp.tile([128, 2 * 63], CD)
                nc.sync.dma_start(out=w9t[:], in_=w9t_d[:])
                b9t = hp.tile([63, 1], F32)
                nc.sync.dma_start(out=b9t[:], in_=b9_d[:])

                if debug:
                    nc.sync.dma_start(out=x1_dbg[:], in_=x1[:])
                    nc.sync.dma_start(out=x2_dbg[:], in_=x2[:])
                    nc.sync.dma_start(out=x3_dbg[:], in_=x3[:])
                xs_ = [x1, x2, x3]
                NC6 = N // 512
                gtmp = hp.tile([128, 8 * NC6], F32)
                for o in range(8):
                    for n in range(NC6):
                        ps6 = hps.tile([128, 512], F32, tag="hps")
                        for kp in range(3):
                            nc.tensor.matmul(
                                ps6[:],
                                w6t[:, kp * 1024 + o * 128: kp * 1024 + (o + 1) * 128],
                                xs_[kp][:, n * 512:(n + 1) * 512],
                                start=(kp == 0), stop=(kp == 2))
                        nc.vector.tensor_reduce(
                            out=gtmp[:, o * NC6 + n: o * NC6 + n + 1],
                            in_=ps6[:], axis=AX.X, op=OP.max)
                g = hp.tile([128, 8], F32)
                nc.vector.tensor_reduce(
                    out=g[:], in_=gtmp.rearrange("p (o n) -> p o n", o=8),
                    axis=AX.X, op=OP.max)
                nc.vector.tensor_tensor(out=g[:], in0=g[:], in1=b6t[:], op=OP.add)
                g2 = hp.tile([128, 8], F32)
                nc.vector.scalar_tensor_tensor(
                    out=g2[:], in0=g[:], scalar=LEAK, in1=g[:],
                    op0=OP.mult, op1=OP.max)
                if debug:
                    nc.sync.dma_start(out=g_dbg[:], in_=g2[:])

                ps7v = hps.tile([128, 4], F32, tag="ps7v", bufs=1)
                for m in range(4):
                    for o in range(8):
                        nc.tensor.matmul(
                            ps7v[:, m:m + 1],
                            w7gt[:, (o * 4 + m) * 128:(o * 4 + m + 1) * 128],
                            g2[:, o:o + 1], start=(o == 0), stop=(o == 7))
                v7 = hp.tile([128, 4], F32)
                nc.vector.tensor_tensor(out=v7[:], in0=ps7v[:], in1=b7t[:], op=OP.add)

                for n in range(NC6):
                    y7 = hsp.tile([128, 4 * 512], CD, tag="y7")
                    for m in range(4):
                        ps7 = hps.tile([128, 512], F32, tag="hps")
                        for kp in range(3):
                            nc.tensor.matmul(
                                ps7[:],
                                w7xt[:, (kp * 4 + m) * 128:(kp * 4 + m + 1) * 128],
                                xs_[kp][:, n * 512:(n + 1) * 512],
                                start=(kp == 0), stop=(kp == 2))
                        t7 = hsp.tile([128, 512], F32, tag="t7")
                        nc.scalar.activation(t7[:], ps7[:], ACTF.Identity,
                                             bias=v7[:, m:m + 1])
                        nc.vector.scalar_tensor_tensor(
                            out=y7[:, m * 512:(m + 1) * 512], in0=t7[:], scalar=LEAK,
                            in1=t7[:], op0=OP.mult, op1=OP.max)
                    y8 = hsp.tile([128, 2 * 512], CD, tag="y8")
                    for m in range(2):
                        ps8 = hps.tile([128, 512], F32, tag="hps")
                        for k in range(4):
                            nc.tensor.matmul(
                                ps8[:], w8t[:, (k * 2 + m) * 128:(k * 2 + m + 1) * 128],
                                y7[:, k * 512:(k + 1) * 512],
                                start=(k == 0), stop=(k == 3))
                        t8 = hsp.tile([128, 512], F32, tag="t7")
                        nc.scalar.activation(t8[:], ps8[:], ACTF.Identity,
                                             bias=b8t[:, m:m + 1])
                        nc.vector.scalar_tensor_tensor(
                            out=y8[:, m * 512:(m + 1) * 512], in0=t8[:], scalar=LEAK,
                            in1=t8[:], op0=OP.mult, op1=OP.max)
                    ps9 = hps.tile([63, 512], F32, tag="hps")
                    for k in range(2):
                        nc.tensor.matmul(ps9[:], w9t[:, k * 63:(k + 1) * 63],
                                         y8[:, k * 512:(k + 1) * 512],
                                         start=(k == 0), stop=(k == 1))
                    o9 = hsp.tile([63, 512], F32, tag="o9")
                    nc.vector.tensor_scalar(o9[:], ps9[:], b9t[:], None, op0=OP.add)
                    nc.sync.dma_start(out=out_d[:, n * 512:(n + 1) * 512], in_=o9[:])

    nc.finalize()
    return nc


# =====================================================================
# Host-side folding
# =====================================================================
def fold_weights(inp):
    """inp: the reference setup_inputs() dict. Returns dict of shared
    (sample-independent) device arrays."""
    def f64(a):
        return np.asarray(a, np.float64)

    out = {}
    W1, s1, b1 = f64(inp["W1"]), f64(inp["s1"]), f64(inp["b1"])
    W1a, W1b = W1[:, :3], W1[:, 3:]
    out["a1w"] = (s1[:, None] * W1a).T.astype(np.float32).copy()
    out["b1w"] = (s1[:, None] * (W1b - W1a)).T.astype(np.float32).copy()
    out["b1"] = b1[:, None].astype(np.float32)
    W2, s2, b2 = f64(inp["W2"]), f64(inp["s2"]), f64(inp["b2"])
    assert (s2 > 0).all()
    out["w2t"] = (s2[:, None] * W2).T.astype(np.float32).copy()
    out["b2"] = b2[:, None].astype(np.float32)
    W3, s3, b3 = f64(inp["W3"]), f64(inp["s3"]), f64(inp["b3"])
    W3a, W3b = W3[:, :64], W3[:, 64:]
    out["a3w"] = (s3[:, None] * W3a).T.astype(np.float32).copy()
    out["b3w"] = (s3[:, None] * (W3b - W3a)).T.astype(np.float32).copy()
    out["b3"] = b3[:, None].astype(np.float32)
    W4, s4, b4 = f64(inp["W4"]), f64(inp["s4"]), f64(inp["b4"])
    assert (s4 > 0).all()
    out["w4t"] = (s4[:, None] * W4).T.astype(np.float32).copy()
    out["b4"] = b4[:, None].astype(np.float32)
    W5, s5, b5 = f64(inp["W5"]), f64(inp["s5"]), f64(inp["b5"])
    W5a, W5b = W5[:, :64], W5[:, 64:]
    out["a5w"] = (s5[:, None] * W5a).T.astype(np.float32).copy()
    out["b5w"] = (s5[:, None] * (W5b - W5a)).T.astype(np.float32).copy()
    out["b5"] = b5[:, None].astype(np.float32)
    W6, s6, b6 = f64(inp["W6"]), f64(inp["s6"]), f64(inp["b6"])
    assert (s6 > 0).all()
    W6f = s6[:, None] * W6
    out["w6t"] = W6f.T.reshape(3, 64, 1024).transpose(1, 0, 2) \
        .reshape(64, 3 * 1024).astype(np.float32).copy()
    out["b6"] = b6.reshape(8, 128).T.astype(np.float32).copy()
    W7, s7, b7 = f64(inp["W7"]), f64(inp["s7"]), f64(inp["b7"])
    W7f = s7[:, None] * W7
    W7g, W7x = W7f[:, :1024], W7f[:, 1024:]
    out["w7gt"] = W7g.T.reshape(8, 128, 4, 128).transpose(1, 0, 2, 3) \
        .reshape(128, -1).astype(np.float32).copy()
    out["b7"] = b7.reshape(4, 128).T.astype(np.float32).copy()
    out["w7xt"] = W7x.T.reshape(3, 64, 4, 128).transpose(1, 0, 2, 3) \
        .reshape(64, -1).astype(np.float32).copy()
    W8, s8, b8 = f64(inp["W8"]), f64(inp["s8"]), f64(inp["b8"])
    W8f = s8[:, None] * W8
    out["w8t"] = W8f.T.reshape(4, 128, 2, 128).transpose(1, 0, 2, 3) \
        .reshape(128, -1).astype(np.float32).copy()
    out["b8"] = b8.reshape(2, 128).T.astype(np.float32).copy()
    out["w9t"] = f64(inp["W9"]).T.reshape(2, 128, 63).transpose(1, 0, 2) \
        .reshape(128, 2 * 63).astype(np.float32).copy()
    out["b9"] = f64(inp["b9"])[:, None].astype(np.float32)
    return out


def fold_sample(sample_x):
    """sample_x: (3, N) float32. Returns per-sample arrays."""
    x = np.asarray(sample_x, np.float64)
    xx = (x * x).sum(0)
    N = x.shape[1]
    return {
        "xr": np.concatenate([x, -0.5 * xx[None, :]], 0).astype(np.float32),
        "xa": np.concatenate([x, np.ones((1, N))], 0).astype(np.float32),
    }


def make_in_maps(inputs, n_cores=4):
    """inputs: reference setup_inputs() dict (numpy). One core per sample."""
    shared = fold_weights(inputs)
    x = np.asarray(inputs["x"])
    in_maps = []
    for c in range(n_cores):
        b = c % x.shape[0]
        m = dict(shared)
        m.update(fold_sample(x[b]))
        in_maps.append(m)
    return in_maps


def cast_inputs(in_maps, nc):
    dts = {}
    for alloc in nc.m.functions[0].allocations:
        if isinstance(alloc, mybir.MemoryLocationSet) and alloc.kind == "ExternalInput":
            dts[alloc.memorylocations[0].name] = mybir.dt.np(alloc.dtype)
    outs = []
    for m in in_maps:
        outs.append({k: np.ascontiguousarray(np.asarray(v).astype(dts[k]))
                     for k, v in m.items() if k in dts})
    return outs


# =====================================================================
# Harness entry point
# =====================================================================
_CACHE = {}


def kernel(**inputs):
    """DGCNN forward. inputs keyed as reference.setup_inputs(); returns
    (B, 63, N) float32. Data-parallel: one NeuronCore per sample."""
    from concourse.bass_utils import run_bass_kernel_spmd

    x = np.asarray(inputs["x"])
    B, _, N = x.shape
    key = (B, N)
    if key not in _CACHE:
        _CACHE[key] = build_core(N=N, conv_dtype="bf16")
    nc = _CACHE[key]
    in_maps = cast_inputs(make_in_maps(inputs, n_cores=B), nc)
    res = run_bass_kernel_spmd(nc, in_maps, core_ids=list(range(B)))
    out = np.stack([res.results[b]["out"] for b in range(B)]).astype(np.float32)
    return out
